# revision 1
# baseline (speedup 1.0000x reference)
"""AttentionConv kernel for Trainium2 (8 NeuronCores, SPMD data-parallel over batch).

Problem: per-channel windowed softmax attention.
  q = wq @ x; k = wk @ pad(x, 3); v = wv @ pad(x, 3)       (1x1 convs = GEMMs)
  s_j[c,w] = q[c,w] * k[c,w+j],  j = 0..6
  out[c,w] = sum_j softmax_j(s)[c,w,j] * v[c,w+j]

Sharding: batch B=8 -> one batch element per core; weights replicated.
Since pad commutes with the channel-mixing GEMM, k/v are computed on the
unpadded x and written into SBUF buffers with 3 zero columns on each side.

Per-core engine mapping:
  TensorE: 3 GEMMs (256x256 @ 256x4096), bf16 in, fp32 PSUM.
  ScalarE: batched PSUM->SBUF evacuation casts, exp (split in j-halves so it
           overlaps the score mults), 1/den via exp(-ln(den)) with Exp+Ln
           pinned to one ACT table set.
  VectorE: windowed score mult, e*v mult, tree adds for num/den (all bf16,
           innermost stride 1 -> 2x DVE mode), final bf16 out = num * rden
           (host upcasts the bf16 output to fp32).
"""

import sys

sys.path.insert(0, "/opt/trn_rl_repo")

import numpy as np

B, C, W = 8, 256, 4096
K7, PAD = 7, 3
WC_G = 1024  # gemm / psum evac group (2 PSUM banks)
WC_A = 2048  # attention chunk

_STATE = {}


def _patch_act_tables():
    """Force Exp and Ln to resolve to the one ACT table set containing both,
    so the kernel pays a single ACT_TABLE_LOAD instead of thrashing."""
    import concourse.bacc as bacc_mod
    import concourse.mybir as mybir
    from concourse.hw_specs import get_activation_tables as orig

    AF = mybir.ActivationFunctionType

    def patched(arch):
        out = {}
        for name, funcs in orig(arch).items():
            f = set(funcs)
            if name != "natural_log_exp_and_others":
                f.discard(AF.Exp)
                f.discard(AF.Ln)
            out[name] = f
        return out

    bacc_mod.get_activation_tables = patched


def _build_nc():
    import concourse.bass as bass
    import concourse.tile as tile
    from concourse import bacc, mybir

    _patch_act_tables()

    bf16 = mybir.dt.bfloat16
    f32 = mybir.dt.float32
    AF = mybir.ActivationFunctionType

    nc = bacc.Bacc("TRN2", target_bir_lowering=False, debug=False, num_devices=8)

    x_d = nc.declare_dram_parameter("x", [C, W], bf16, isOutput=False)
    w_d = {
        t: nc.declare_dram_parameter(f"wt{t}", [C, C], bf16, isOutput=False)
        for t in "qkv"
    }
    out_d = nc.declare_dram_parameter("out", [C, W], bf16, isOutput=True)

    WP = W + 2 * PAD  # padded width for k/v
    n_ag = W // WC_G  # gemm groups per co block
    n_ac = W // WC_A  # attention chunks per co block

    with tile.TileContext(nc) as tc:
        from contextlib import ExitStack

        with ExitStack() as ctx:
            persist = ctx.enter_context(tc.tile_pool(name="persist", bufs=1))
            psum = ctx.enter_context(tc.tile_pool(name="psum", bufs=3, space="PSUM"))
            spool = ctx.enter_context(tc.tile_pool(name="spool", bufs=3))
            dpool = ctx.enter_context(tc.tile_pool(name="dpool", bufs=2))
            opool = ctx.enter_context(tc.tile_pool(name="opool", bufs=2))

            # ---- persistent SBUF tensors ----
            xb = persist.tile([128, 2, W], bf16, tag="xb")  # x, ci-major blocks
            wsb = {
                t: persist.tile([128, 2, C], bf16, name=f"wsb_{t}", tag=f"wsb_{t}")
                for t in "qkv"
            }  # w.T
            qsb = persist.tile([128, 2, W], bf16, tag="qsb")
            ksb = persist.tile([128, 2, WP], bf16, tag="ksb")
            vsb = persist.tile([128, 2, WP], bf16, tag="vsb")

            # ---- loads ----
            # wq first (feeds the PE warmup), then x (critical path), then wk/wv.
            for cb in range(2):
                nc.sync.dma_start(
                    out=wsb["q"][:, cb, :], in_=w_d["q"][cb * 128 : (cb + 1) * 128, :]
                )
            for cb in range(2):
                nc.sync.dma_start(
                    out=xb[:, cb, :], in_=x_d[cb * 128 : (cb + 1) * 128, :]
                )
            for t in "kv":
                for cb in range(2):
                    nc.sync.dma_start(
                        out=wsb[t][:, cb, :], in_=w_d[t][cb * 128 : (cb + 1) * 128, :]
                    )

            # zero the pad columns of k and v
            for buf in (ksb, vsb):
                for cb in range(2):
                    nc.vector.memset(buf[:, cb, 0:PAD], 0.0)
                    nc.vector.memset(buf[:, cb, W + PAD : WP], 0.0)

            # PE warmup burst: ~7us of dummy matmuls on the wq tiles so the
            # HAM clock-gate releases before the real GEMM stream arrives.
            wps = psum.tile([128, WC_G], f32, name="wps", tag="ps")
            for i in range(28):
                nc.tensor.matmul(
                    wps[:, 0:256],
                    wsb["q"][:, 0, 0:128],
                    wsb["q"][:, i % 2, :],
                    start=True,
                    stop=True,
                    skip_group_check=True,
                )

            def gemm_group(co, g):
                """q/k/v GEMM for output cols [g*WC_G, (g+1)*WC_G) of co-block,
                batched into one PSUM tile + one ACT evacuation per tensor."""
                co_sl = slice(co * 128, (co + 1) * 128)
                for t in "qkv":
                    ps = psum.tile([128, WC_G], f32, name="ps", tag="ps")
                    for i in range(WC_G // 512):
                        w0 = g * WC_G + i * 512
                        for ci in range(2):
                            nc.tensor.matmul(
                                ps[:, i * 512 : (i + 1) * 512],
                                wsb[t][:, ci, co_sl],
                                xb[:, ci, w0 : w0 + 512],
                                start=(ci == 0),
                                stop=(ci == 1),
                            )
                    if t == "q":
                        dst = qsb[:, co, g * WC_G : (g + 1) * WC_G]
                    else:
                        buf = ksb if t == "k" else vsb
                        dst = buf[:, co, PAD + g * WC_G : PAD + (g + 1) * WC_G]
                    nc.scalar.copy(out=dst, in_=ps[:, :])

            def att_scores(co, ai):
                """scores + exp for att chunk (co, ai); returns the e tile.
                Split into j-halves so exp starts before all scores finish."""
                w0 = ai * WC_A
                s = spool.tile([128, K7, WC_A], bf16, name="s", tag="s")

                qsl = qsb[:, co, w0 : w0 + WC_A]
                ksl = ksb[:, co, w0 : w0 + WC_A]

                def q_bc(n):
                    return bass.AP(
                        tensor=qsl.tensor,
                        offset=qsl.offset,
                        ap=[qsl.ap[0], [0, n], [1, WC_A]],
                    )

                def k_wn(j0, n):
                    return bass.AP(
                        tensor=ksl.tensor,
                        offset=ksl.offset + j0,
                        ap=[ksl.ap[0], [1, n], [1, WC_A]],
                    )

                # scores then e = exp(s) in place, in two j-halves
                nc.vector.tensor_mul(s[:, 0:4, :], q_bc(4), k_wn(0, 4))
                nc.scalar.activation(s[:, 0:4, :], s[:, 0:4, :], AF.Exp)
                nc.vector.tensor_mul(s[:, 4:7, :], q_bc(3), k_wn(4, 3))
                nc.scalar.activation(s[:, 4:7, :], s[:, 4:7, :], AF.Exp)
                return s

            def att_rest(co, ai, s):
                """softmax-normalize + v-window weighted sum + store."""
                w0 = ai * WC_A
                co_sl = slice(co * 128, (co + 1) * 128)
                dent = dpool.tile([128, 3, WC_A], bf16, name="dent", tag="dent")
                vsl = vsb[:, co, w0 : w0 + WC_A]
                v_w = bass.AP(
                    tensor=vsl.tensor,
                    offset=vsl.offset,
                    ap=[vsl.ap[0], [1, K7], [1, WC_A]],
                )
                # den tree -> dent[:, 0, :]
                # pairs (e0+e1, e2+e3) need only the first exp half
                s02 = bass.AP(
                    tensor=s.tensor,
                    offset=s.offset,
                    ap=[s.ap[0], [2 * WC_A, 2], [1, WC_A]],
                )
                s13 = bass.AP(
                    tensor=s.tensor,
                    offset=s.offset + WC_A,
                    ap=[s.ap[0], [2 * WC_A, 2], [1, WC_A]],
                )
                nc.vector.tensor_add(dent[:, 0:2, :], s02, s13)
                nc.vector.tensor_add(dent[:, 2, :], s[:, 4, :], s[:, 5, :])
                nc.vector.tensor_add(dent[:, 0, :], dent[:, 0, :], dent[:, 1, :])
                nc.vector.tensor_add(dent[:, 0, :], dent[:, 0, :], dent[:, 2, :])
                nc.vector.tensor_add(dent[:, 0, :], dent[:, 0, :], s[:, 6, :])
                # rden = exp(-ln(den)) -> dent[:, 1, :]
                nc.scalar.activation(dent[:, 1, :], dent[:, 0, :], AF.Ln)
                nc.scalar.activation(dent[:, 1, :], dent[:, 1, :], AF.Exp, scale=-1.0)
                # ev = e * v_shift, in place; num tree -> s[:, 0, :]
                nc.vector.tensor_mul(s[:, :, :], s[:, :, :], v_w)
                nc.vector.tensor_add(s[:, 0:3, :], s[:, 0:3, :], s[:, 3:6, :])
                nc.vector.tensor_add(s[:, 0, :], s[:, 0, :], s[:, 1, :])
                nc.vector.tensor_add(s[:, 0, :], s[:, 0, :], s[:, 2, :])
                nc.vector.tensor_add(s[:, 0, :], s[:, 0, :], s[:, 6, :])
                # out = num * rden (bf16, 2x mode); host upcasts to fp32
                oc = opool.tile([128, WC_A], bf16, name="oc", tag="oc")
                nc.vector.tensor_mul(oc[:, :], s[:, 0, :], dent[:, 1, :])
                nc.sync.dma_start(out=out_d[co_sl, w0 : w0 + WC_A], in_=oc[:, :])

            gpg = WC_A // WC_G  # gemm groups per attention chunk
            chunks = [(co, ai) for co in range(2) for ai in range(n_ac)]
            emitted = [0, 0]  # gemm groups emitted per co block

            def need_gemms(co, ai):
                hi = min((ai + 1) * gpg + 1, n_ag)
                for g in range(emitted[co], hi):
                    gemm_group(co, g)
                emitted[co] = max(emitted[co], hi)

            for ch in chunks:
                need_gemms(*ch)
                att_rest(*ch, att_scores(*ch))

    nc.finalize()
    return nc


def _get_nc():
    if "nc" not in _STATE:
        _STATE["nc"] = _build_nc()
    return _STATE["nc"]


def kernel(x, wq, wk, wv):
    import ml_dtypes

    bf = ml_dtypes.bfloat16
    nc = _get_nc()

    x = np.asarray(x, dtype=np.float32)
    wqT = np.ascontiguousarray(np.asarray(wq, dtype=np.float32).T).astype(bf)
    wkT = np.ascontiguousarray(np.asarray(wk, dtype=np.float32).T).astype(bf)
    wvT = np.ascontiguousarray(np.asarray(wv, dtype=np.float32).T).astype(bf)
    xb = x.astype(bf)

    in_maps = [
        {
            "x": np.ascontiguousarray(xb[b]),
            "wtq": wqT,
            "wtk": wkT,
            "wtv": wvT,
        }
        for b in range(B)
    ]

    from concourse.bass_utils import run_bass_kernel_spmd

    res = run_bass_kernel_spmd(nc, in_maps, core_ids=list(range(B)))
    outs = [np.asarray(res.results[i]["out"], dtype=np.float32) for i in range(B)]
    return np.stack(outs)



# revision 7
# speedup vs baseline: 1.3089x; 1.3089x over previous
"""AttentionConv kernel for Trainium2 (8 NeuronCores, SPMD data-parallel over batch).

Problem: per-channel windowed softmax attention.
  q = wq @ x; k = wk @ pad(x, 3); v = wv @ pad(x, 3)       (1x1 convs = GEMMs)
  s_j[c,w] = q[c,w] * k[c,w+j],  j = 0..6
  out[c,w] = sum_j softmax_j(s)[c,w,j] * v[c,w+j]

Sharding: batch B=8 -> one batch element per core; weights replicated.

v2 engine mapping (vs v1 which ran everything elementwise on DVE/ACT):
  TensorE: q/k/v GEMMs (bf16) AND the two 7-plane window reductions
           (den = sum_j e_j, num = sum_j e_j*v_j) as identity-weight
           matmuls accumulating in PSUM fp32. Keeps PE busy -> HAM stays
           at 8/8 (2.4 GHz) instead of the 4/8 throttle v1 suffered.
  VectorE: score mults and e*v mults (bf16 2x, full-width 4096 rows to
           amortize the ~235-cycle per-row bubble), exp via Schraudolph
           bit-trick tensor_scalar -> int16 (4x mode), final
           out = num(PSUM) * rden.
  ScalarE: PSUM->SBUF GEMM evacuation casts, optional exact exp planes,
           rden = exp(-ln(den)) pinned to the one ACT table set with both.
  Host upcasts the bf16 output to fp32.
"""

import sys

sys.path.insert(0, "/opt/trn_rl_repo")

import numpy as np

B, C, W = 8, 256, 4096
K7, PAD = 7, 3
WP = W + 2 * PAD
GG = 1024  # gemm / psum evac group (2 PSUM banks)
SC = 1024  # sum-chunk width for den/num PSUM accumulators

# --- tuning knobs -----------------------------------------------------------
N_ACT_PLANES = 0  # j-planes [0, n) get exact ACT exp; rest Schraudolph on DVE
SCHRAUD_C0 = 184.6650390625  # 2^7 / ln 2
SCHRAUD_C1 = 16250.0  # 127 * 128 - sigma
SUMS_ON_PE = True  # identity-matmul PSUM sums (else DVE trees)
EV_POOL_PLANES = 0  # ev-mult planes given to gpsimd (0 = all DVE)

_STATE = {}


def _patch_act_tables():
    """Force Exp and Ln to resolve to the one ACT table set containing both,
    so the kernel pays a single ACT_TABLE_LOAD instead of thrashing."""
    import concourse.bacc as bacc_mod
    import concourse.mybir as mybir
    from concourse.hw_specs import get_activation_tables as orig

    AF = mybir.ActivationFunctionType

    def patched(arch):
        out = {}
        for name, funcs in orig(arch).items():
            f = set(funcs)
            if name != "natural_log_exp_and_others":
                f.discard(AF.Exp)
                f.discard(AF.Ln)
            out[name] = f
        return out

    bacc_mod.get_activation_tables = patched


def _build_nc():
    import concourse.bass as bass
    import concourse.tile as tile
    from concourse import bacc, mybir

    _patch_act_tables()

    bf16 = mybir.dt.bfloat16
    i16 = mybir.dt.int16
    f32 = mybir.dt.float32
    AF = mybir.ActivationFunctionType
    ALU = mybir.AluOpType

    nc = bacc.Bacc("TRN2", target_bir_lowering=False, debug=False, num_devices=8)

    x_d = nc.declare_dram_parameter("x", [C, W], bf16, isOutput=False)
    w_d = {
        t: nc.declare_dram_parameter(f"wt{t}", [C, C], bf16, isOutput=False)
        for t in "qkv"
    }
    id_d = nc.declare_dram_parameter("ident", [128, 128], bf16, isOutput=False)
    out_d = nc.declare_dram_parameter("out", [C, W], bf16, isOutput=True)

    n_gg = W // GG  # gemm groups per co block (4)
    n_sc = W // SC  # sum chunks per co block (4)

    with tile.TileContext(nc) as tc:
        from contextlib import ExitStack

        with ExitStack() as ctx:
            persist = ctx.enter_context(tc.tile_pool(name="persist", bufs=1))
            lpool = ctx.enter_context(tc.tile_pool(name="lpool", bufs=2))
            rpool = ctx.enter_context(tc.tile_pool(name="rpool", bufs=3))
            opool = ctx.enter_context(tc.tile_pool(name="opool", bufs=3))

            # ---- persistent SBUF tensors ----
            xb = persist.tile([128, 2, W], bf16, tag="xb")  # x, ci-major blocks
            wsb = {
                t: persist.tile([128, 2, C], bf16, name=f"wsb_{t}", tag=f"wsb_{t}")
                for t in "qkv"
            }  # w.T
            idt = persist.tile([128, 128], bf16, tag="idt")
            qsb = persist.tile([128, 2, W], bf16, tag="qsb")
            ksb = persist.tile([128, 2, WP], bf16, tag="ksb")
            vsb = persist.tile([128, 2, WP], bf16, tag="vsb")
            # score/e/ev planes, both co blocks (ev done in place over e)
            st = persist.tile([128, 2, K7, W], bf16, tag="st")

            # ---- loads ----
            for cb in range(2):
                nc.sync.dma_start(
                    out=wsb["q"][:, cb, :], in_=w_d["q"][cb * 128 : (cb + 1) * 128, :]
                )
            nc.sync.dma_start(out=idt[:, :], in_=id_d[:, :])
            for cb in range(2):
                nc.sync.dma_start(
                    out=xb[:, cb, :], in_=x_d[cb * 128 : (cb + 1) * 128, :]
                )
            for t in "kv":
                for cb in range(2):
                    nc.sync.dma_start(
                        out=wsb[t][:, cb, :], in_=w_d[t][cb * 128 : (cb + 1) * 128, :]
                    )

            # zero the pad columns of k and v
            for buf in (ksb, vsb):
                for cb in range(2):
                    nc.vector.memset(buf[:, cb, 0:PAD], 0.0)
                    nc.vector.memset(buf[:, cb, W + PAD : WP], 0.0)

            def warmup(gpsum):
                """PE warmup burst: dummy matmuls on the wq tiles so the pstate
                ramp + HAM clock-gate release before the real GEMM stream."""
                wps = gpsum.tile([128, GG], f32, name="wps", tag="gps")
                for i in range(16):
                    nc.tensor.matmul(
                        wps[:, 0:256],
                        wsb["q"][:, 0, 0:128],
                        wsb["q"][:, i % 2, :],
                        start=True,
                        stop=True,
                        skip_group_check=True,
                    )

            def gemm_group(co, g, gpsum):
                """q/k/v GEMM for output cols [g*GG, (g+1)*GG) of co-block,
                one PSUM tile + one ACT evacuation per tensor."""
                co_sl = slice(co * 128, (co + 1) * 128)
                for t in "qkv":
                    ps = gpsum.tile([128, GG], f32, name="ps", tag="gps")
                    for i in range(GG // 512):
                        w0 = g * GG + i * 512
                        for ci in range(2):
                            nc.tensor.matmul(
                                ps[:, i * 512 : (i + 1) * 512],
                                wsb[t][:, ci, co_sl],
                                xb[:, ci, w0 : w0 + 512],
                                start=(ci == 0),
                                stop=(ci == 1),
                            )
                    if t == "q":
                        dst = qsb[:, co, g * GG : (g + 1) * GG]
                    else:
                        buf = ksb if t == "k" else vsb
                        dst = buf[:, co, PAD + g * GG : PAD + (g + 1) * GG]
                    nc.scalar.copy(out=dst, in_=ps[:, :])

            def q_bc(co, j0, nj):
                qsl = qsb[:, co, :]
                return bass.AP(
                    tensor=qsl.tensor,
                    offset=qsl.offset,
                    ap=[qsl.ap[0], [0, nj], [1, W]],
                )

            def k_win(co, j0, nj):
                ksl = ksb[:, co, :]
                return bass.AP(
                    tensor=ksl.tensor,
                    offset=ksl.offset + j0,
                    ap=[ksl.ap[0], [1, nj], [1, W]],
                )

            def v_win(co, j0, nj):
                vsl = vsb[:, co, :]
                return bass.AP(
                    tensor=vsl.tensor,
                    offset=vsl.offset + j0,
                    ap=[vsl.ap[0], [1, nj], [1, W]],
                )

            def scores_exp(co):
                """s_j = q*k_j then e = exp(s) in place, split in j-halves so
                exp overlaps the second score mult."""
                for j0, nj in ((0, 4), (4, 3)):
                    dst = st[:, co, j0 : j0 + nj, :]
                    nc.vector.tensor_tensor(
                        dst, q_bc(co, j0, nj), k_win(co, j0, nj), ALU.mult
                    )
                    a_lo = max(j0, 0)
                    a_hi = min(j0 + nj, N_ACT_PLANES)
                    if a_hi > a_lo:  # exact ACT planes
                        sl = st[:, co, a_lo:a_hi, :]
                        nc.scalar.activation(sl, sl, AF.Exp)
                    s_lo = max(j0, N_ACT_PLANES)
                    s_hi = j0 + nj
                    if s_hi > s_lo:  # Schraudolph planes on DVE (4x)
                        sl = st[:, co, s_lo:s_hi, :]
                        nc.vector.tensor_scalar(
                            sl.bitcast(i16),
                            sl,
                            SCHRAUD_C0,
                            SCHRAUD_C1,
                            ALU.mult,
                            ALU.add,
                        )

            def ev_mult(co):
                """ev_j = e_j * v_j in place (PE den sums must already have
                consumed the e values for this co block)."""
                n_dve = K7 - EV_POOL_PLANES
                if n_dve > 0:
                    sl = st[:, co, 0:n_dve, :]
                    nc.vector.tensor_tensor(sl, sl, v_win(co, 0, n_dve), ALU.mult)
                if EV_POOL_PLANES > 0:
                    sl = st[:, co, n_dve:K7, :]
                    nc.gpsimd.tensor_tensor(
                        sl, sl, v_win(co, n_dve, EV_POOL_PLANES), ALU.mult
                    )

            def pe_sum(co, m, pool, tag):
                """7-plane sum over j for w-cols [m*SC, (m+1)*SC) via identity
                matmuls accumulating in PSUM. Returns the PSUM tile."""
                ps = pool.tile([128, SC], f32, name=tag, tag=tag)
                for h in range(SC // 512):
                    w0 = m * SC + h * 512
                    for j in range(K7):
                        nc.tensor.matmul(
                            ps[:, h * 512 : (h + 1) * 512],
                            idt[:, :],
                            st[:, co, j, w0 : w0 + 512],
                            start=(j == 0),
                            stop=(j == K7 - 1),
                        )
                return ps

            def rden_of(denp):
                """rden = exp(-ln(den)) on ACT; ln kept fp32 to avoid bf16
                ulp noise on large |ln den|."""
                t = lpool.tile([128, SC], f32, name="lnt", tag="lnt")
                r = rpool.tile([128, SC], bf16, name="rd", tag="rd")
                nc.scalar.activation(t[:, :], denp[:, :], AF.Ln)
                nc.scalar.activation(r[:, :], t[:, :], AF.Exp, scale=-1.0)
                return r

            def final_out(co, m, nump, rd):
                """out = num (PSUM fp32) * rden -> bf16, DMA to HBM."""
                co_sl = slice(co * 128, (co + 1) * 128)
                w0 = m * SC
                oc = opool.tile([128, SC], bf16, name="oc", tag="oc")
                nc.vector.tensor_tensor(oc[:, :], nump[:, :], rd[:, :], ALU.mult)
                nc.sync.dma_start(out=out_d[co_sl, w0 : w0 + SC], in_=oc[:, :])

            # ---- emission ----
            # GEMM phase with its own (scoped) PSUM pool; releasing it frees
            # the banks for the den/num accumulators of the attention phase.
            with tc.tile_pool(name="gpsum", bufs=3, space="PSUM") as gpsum:
                warmup(gpsum)
                for co in range(2):
                    for g in range(n_gg):
                        gemm_group(co, g, gpsum)
            for co in range(2):
                scores_exp(co)
            with (
                tc.tile_pool(name="dpsum", bufs=2, space="PSUM") as dpsum,
                tc.tile_pool(name="npsum", bufs=2, space="PSUM") as npsum,
            ):
                dens = {}
                for co in range(2):
                    for m in range(n_sc):
                        dens[(co, m)] = pe_sum(co, m, dpsum, "den")
                for co in range(2):
                    ev_mult(co)
                for co in range(2):
                    for m in range(n_sc):
                        rd = rden_of(dens[(co, m)])  # ACT; frees den tile
                        nump = pe_sum(co, m, npsum, "num")
                        final_out(co, m, nump, rd)

    nc.finalize()
    return nc


def _get_nc():
    if "nc" not in _STATE:
        _STATE["nc"] = _build_nc()
    return _STATE["nc"]


def _make_in_maps(x, wq, wk, wv):
    import ml_dtypes

    bf = ml_dtypes.bfloat16

    x = np.asarray(x, dtype=np.float32)
    wqT = np.ascontiguousarray(np.asarray(wq, dtype=np.float32).T).astype(bf)
    wkT = np.ascontiguousarray(np.asarray(wk, dtype=np.float32).T).astype(bf)
    wvT = np.ascontiguousarray(np.asarray(wv, dtype=np.float32).T).astype(bf)
    xb = x.astype(bf)
    ident = np.eye(128, dtype=np.float32).astype(bf)

    return [
        {
            "x": np.ascontiguousarray(xb[b]),
            "wtq": wqT,
            "wtk": wkT,
            "wtv": wvT,
            "ident": ident,
        }
        for b in range(B)
    ]


def kernel(x, wq, wk, wv):
    nc = _get_nc()
    in_maps = _make_in_maps(x, wq, wk, wv)

    from concourse.bass_utils import run_bass_kernel_spmd

    res = run_bass_kernel_spmd(nc, in_maps, core_ids=list(range(B)))
    outs = [np.asarray(res.results[i]["out"], dtype=np.float32) for i in range(B)]
    return np.stack(outs)
